# revision 34
# baseline (speedup 1.0000x reference)
"""Multi-Head Latent Attention (GQA, causal) on 8 Trainium2 NeuronCores.

Sharding: tensor-parallel by heads. Core c owns query heads 4c..4c+3 and
kv head c. Each core computes:
  - its slice of the q projection (output dims c*512..(c+1)*512),
  - its S/8 sequence shard of the kc/vc down-projections, AllGathered so
    every core sees the full compressed latents (the reference's scrambled
    latent reshape is folded into strided access patterns),
  - its kv head's up-projections (k in [d,t] layout, v directly in [t,d]),
  - head-parallel causal attention computed TRANSPOSED: scores^T[t,q] come
    straight out of the PE in the layout the AV matmul consumes, so no
    per-block P transposes are needed. The AV matmul's 129th column (a
    constant-ones column of V) yields the softmax denominator; the
    normalization is a per-partition scale on the AV output copy, and one
    128x128 PE transpose per (head, q-tile) restores [d,q] for o_proj,
  - a partial o-projection (input dims c*512..(c+1)*512) over the full
    hidden size, interleaved into the NEXT chunk's attention loop so the
    PE has filler work during exp-bound stretches.
The host sums the 8 partial outputs (the all-reduce after o_proj).
"""

import sys

import ml_dtypes
import numpy as np

if "/opt/trn_rl_repo" not in sys.path:
    sys.path.insert(0, "/opt/trn_rl_repo")

B, S, HID = 1, 2048, 4096
H, HK, D = 32, 8, 128
L = D // 4  # 32
NCORE = 8
HPC = H // NCORE  # 4 query heads per core
NKT = HID // 128  # 32 k-tiles over hidden dim
CHUNK = 512
NCHUNK = S // CHUNK  # 4
NSQ = S // 128  # 16 sq tiles
SSH = S // NCORE  # 256
NEG = -1e9

_BUILT = None


def _build():
    import concourse.mybir as mybir
    import concourse.tile as tile
    from concourse import bacc

    f32 = mybir.dt.float32
    bf16 = mybir.dt.bfloat16
    EXP = mybir.ActivationFunctionType.Exp

    nc = bacc.Bacc()

    ht = nc.dram_tensor("ht", [HID, S], bf16, kind="ExternalInput")
    htm = nc.dram_tensor("htm", [HID, SSH], bf16, kind="ExternalInput")
    wqt = nc.dram_tensor("wqt", [HID, HPC * D], bf16, kind="ExternalInput")
    wkdt = nc.dram_tensor("wkdt", [HID, HK * L], bf16, kind="ExternalInput")
    wvdt = nc.dram_tensor("wvdt", [HID, HK * L], bf16, kind="ExternalInput")
    wkup = nc.dram_tensor("wkup", [128, 8 * D], bf16, kind="ExternalInput")
    wvup = nc.dram_tensor("wvup", [128, 8 * D], bf16, kind="ExternalInput")
    wot = nc.dram_tensor("wot", [HPC * D, HID], bf16, kind="ExternalInput")
    maskt = nc.dram_tensor("maskt", [128, 128], f32, kind="ExternalInput")
    ident = nc.dram_tensor("ident", [128, 128], bf16, kind="ExternalInput")
    outp = nc.dram_tensor("out", [S, HID], bf16, kind="ExternalOutput")
    # kc/vc shard exchange: [p, tgt*512 + m*256 + u] per core -> gathered
    cv_bounce = nc.dram_tensor("cv_bounce", [128, 1024], bf16)
    cv_gath = nc.dram_tensor("cv_gath", [NCORE, 128, 1024], bf16, addr_space="Shared")

    with tile.TileContext(nc) as tc:
        with (
            tc.tile_pool(name="weights", bufs=1) as wpool,
            tc.tile_pool(name="persist", bufs=1) as ppool,
            tc.tile_pool(name="stream", bufs=8) as spool,
            tc.tile_pool(name="outs", bufs=5) as opool,
        ):
            # ---- constants + resident weights (most loads are deferred
            #      behind the cv bounce to keep the early DMA window clear) ----
            ones_sb = wpool.tile([128, 1], bf16)
            nc.gpsimd.memset(ones_sb[:], 1.0)
            mask_sb = wpool.tile([128, 128], f32)
            id_sb = wpool.tile([128, 128], bf16)
            wkup_sb = wpool.tile([128, 8 * D], bf16)
            wvup_sb = wpool.tile([128, 8 * D], bf16)
            wq_sb = wpool.tile([128, NKT, HPC * D], bf16)
            wq_r = wqt.rearrange("(k p) c -> p k c", p=128)
            qg = [0, 2, 4, 8, 12, 16, 20, 24, 28, 32]
            for lo, hi in zip(qg, qg[1:]):
                ks = slice(lo, hi)
                nc.scalar.dma_start(out=wq_sb[:, ks, :], in_=wq_r[:, ks, :])
            wo_sb = wpool.tile([128, HPC, HID], bf16)
            wo_r = wot.rearrange("(k p) c -> p k c", p=128)

            # ---- persistent activations ----
            qT = ppool.tile([128, HPC, S], bf16)  # [d, head, s]
            kcT = ppool.tile([128, 2, S], bf16)  # [latent%128, latent//128, s]
            vcT = ppool.tile([128, 2, S], bf16)
            kT = ppool.tile([128, S], bf16)  # [d, t] for our kv head
            # [t%128, t//128, d]; col 128 is a constant 1.0 column so the
            # AV matmul also produces the softmax denominator (col 128 of out)
            v_sb = ppool.tile([128, NSQ, 132], bf16)
            nc.gpsimd.memset(v_sb[:, :, 128:129], 1.0)

            ht_r = ht.rearrange("(k p) s -> p k s", p=128)

            # ---- phases B0+B: q chunk 0 runs FIRST on a clean DMA feed
            #      (PE ramps to full clock); the cv seq-shard follows once its
            #      weights have streamed in behind chunk 0, so the bounce +
            #      AllGather fly during chunks 1-3 ----
            with tc.tile_pool(name="psq", bufs=1, space="PSUM") as psq:
                with (
                    tc.tile_pool(name="b0", bufs=1) as bpool,
                    tc.tile_pool(name="psb0", bufs=1, space="PSUM") as psb0,
                ):
                    wkd_sb = bpool.tile([128, NKT, HK * L], bf16)
                    wvd_sb = bpool.tile([128, NKT, HK * L], bf16)
                    wkd_r = wkdt.rearrange("(k p) c -> p k c", p=128)
                    wvd_r = wvdt.rearrange("(k p) c -> p k c", p=128)
                    hm = bpool.tile([128, NKT, SSH], bf16)
                    hm_r = htm.rearrange("(k p) s -> p k s", p=128)
                    for g in range(8):
                        ks = slice(g * 4, (g + 1) * 4)
                        nc.sync.dma_start(out=wkd_sb[:, ks, :], in_=wkd_r[:, ks, :])
                        nc.sync.dma_start(out=wvd_sb[:, ks, :], in_=wvd_r[:, ks, :])
                        if g % 2 == 1:
                            hs_ = slice((g // 2) * 8, (g // 2 + 1) * 8)
                            nc.sync.dma_start(out=hm[:, hs_, :], in_=hm_r[:, hs_, :])
                    # q chunk 0
                    ps_q = [
                        psq.tile([128, CHUNK], f32, tag=f"ps_q{m}", name=f"ps_q{m}")
                        for m in range(HPC)
                    ]
                    for kp in range(NKT // 2):
                        hch = spool.tile(
                            [128, 2, CHUNK], bf16, tag="hch", name="hch"
                        )
                        nc.gpsimd.dma_start(
                            out=hch[:], in_=ht_r[:, 2 * kp : 2 * kp + 2, 0:CHUNK]
                        )
                        for kk in range(2):
                            k = 2 * kp + kk
                            st = dict(start=(k == 0), stop=(k == NKT - 1))
                            for m in range(HPC):
                                nc.tensor.matmul(
                                    ps_q[m][:],
                                    lhsT=wq_sb[:, k, m * 128 : (m + 1) * 128],
                                    rhs=hch[:, kk, :],
                                    **st,
                                )
                    # cv seq-shard; a couple of q chunk-1 k-tile pairs are
                    # interleaved at the points where the cv weight feed lags
                    # (reusing chunk 0's accumulators, free by then)
                    ps_cv = [
                        psb0.tile([128, SSH], f32, tag=f"ps_cv{t}", name=f"ps_cv{t}")
                        for t in range(4)
                    ]
                    ps_q1 = [
                        psq.tile([128, CHUNK], f32, tag=f"ps_q{m}", name=f"ps_q{m}")
                        for m in range(HPC)
                    ]
                    for k in range(NKT):
                        for ti, wsb_d in ((0, wkd_sb), (1, wvd_sb)):
                            for m in range(2):
                                nc.tensor.matmul(
                                    ps_cv[ti * 2 + m][:],
                                    lhsT=wsb_d[:, k, m * 128 : (m + 1) * 128],
                                    rhs=hm[:, k, :],
                                    start=(k == 0),
                                    stop=(k == NKT - 1),
                                )
                        if k in (11, 21):
                            kp = (k - 11) // 10
                            hch = spool.tile(
                                [128, 2, CHUNK], bf16, tag="hch", name="hch"
                            )
                            nc.gpsimd.dma_start(
                                out=hch[:],
                                in_=ht_r[:, 2 * kp : 2 * kp + 2, CHUNK : 2 * CHUNK],
                            )
                            for kk in range(2):
                                kq = 2 * kp + kk
                                for m in range(HPC):
                                    nc.tensor.matmul(
                                        ps_q1[m][:],
                                        lhsT=wq_sb[:, kq, m * 128 : (m + 1) * 128],
                                        rhs=hch[:, kk, :],
                                        start=(kq == 0),
                                        stop=False,
                                    )
                    cvst = bpool.tile([128, 1024], bf16)
                    for t in range(4):
                        ti, m = t // 2, t % 2
                        (nc.vector.tensor_copy if t % 2 == 0 else nc.scalar.copy)(
                            cvst[:, ti * 512 + m * 256 : ti * 512 + (m + 1) * 256],
                            ps_cv[t][:],
                        )
                    nc.sync.dma_start(out=cv_bounce[:], in_=cvst[:])
                    # chunk 0's qT copies are not needed until attention;
                    # keep them off the copy queues until the bounce is away
                    for m in range(HPC):
                        (nc.scalar.copy if m % 2 else nc.vector.tensor_copy)(
                            qT[:, m, 0:CHUNK], ps_q[m][:]
                        )
                    # attention-phase constants behind the bounce
                    nc.sync.dma_start(out=mask_sb[:], in_=maskt[:])
                    nc.sync.dma_start(out=id_sb[:], in_=ident[:])
                    nc.sync.dma_start(out=wkup_sb[:], in_=wkup[:])
                    nc.sync.dma_start(out=wvup_sb[:], in_=wvup[:])

                nc.gpsimd.collective_compute(
                    "AllGather",
                    mybir.AluOpType.bypass,
                    replica_groups=[list(range(NCORE))],
                    ins=[cv_bounce[:]],
                    outs=[cv_gath[:]],
                )
                g_r = cv_gath.rearrange("r p (t m u) -> t p m r u", t=2, m=2)
                for m in range(2):
                    nc.sync.dma_start(
                        out=kcT[:, m, :].rearrange("p (r u) -> p r u", r=NCORE),
                        in_=g_r[0, :, m],
                    )
                    nc.sync.dma_start(
                        out=vcT[:, m, :].rearrange("p (r u) -> p r u", r=NCORE),
                        in_=g_r[1, :, m],
                    )
                # wo behind the gathers on the sync queue
                for g in range(HPC):
                    nc.sync.dma_start(out=wo_sb[:, g, :], in_=wo_r[:, g, :])

                # q chunks 1-3
                with tc.tile_pool(name="psq2", bufs=1, space="PSUM") as psq2:
                    for sc in range(1, NCHUNK):
                        if sc == 1:
                            ps_q = ps_q1
                        else:
                            pool = psq2 if sc % 2 == 0 else psq
                            ps_q = [
                                pool.tile(
                                    [128, CHUNK], f32, tag=f"ps_q{m}", name=f"ps_q{m}"
                                )
                                for m in range(HPC)
                            ]
                        for kp in range(2 if sc == 1 else 0, NKT // 2):
                            hch = spool.tile(
                                [128, 2, CHUNK], bf16, tag="hch", name="hch"
                            )
                            nc.gpsimd.dma_start(
                                out=hch[:],
                                in_=ht_r[
                                    :,
                                    2 * kp : 2 * kp + 2,
                                    sc * CHUNK : (sc + 1) * CHUNK,
                                ],
                            )
                            for kk in range(2):
                                k = 2 * kp + kk
                                st = dict(start=(k == 0), stop=(k == NKT - 1))
                                for m in range(HPC):
                                    nc.tensor.matmul(
                                        ps_q[m][:],
                                        lhsT=wq_sb[:, k, m * 128 : (m + 1) * 128],
                                        rhs=hch[:, kk, :],
                                        **st,
                                    )
                        cs = slice(sc * CHUNK, (sc + 1) * CHUNK)
                        for m in range(HPC):
                            (nc.scalar.copy if m % 2 else nc.vector.tensor_copy)(
                                qT[:, m, cs], ps_q[m][:]
                            )

            # k_cmp[t, c'] with t = h*256+u, c' = r*64 + half*32 + j maps to
            #   (half==0 ? KC : VC)[8u + r (+4 for v_cmp), h*32 + j]
            # so the latent operand is a stride-8 slice of kcT/vcT along seq.
            kc_r = kcT.rearrange("p m (u r) -> p m r u", r=8)
            vc_r = vcT.rearrange("p m (u r) -> p m r u", r=8)
            with (
                tc.tile_pool(name="pt", bufs=1) as ptpool,
                tc.tile_pool(name="attn", bufs=3) as apool,
                tc.tile_pool(name="pss", bufs=3, space="PSUM") as pss,
                tc.tile_pool(name="psav", bufs=2, space="PSUM") as psav,
                tc.tile_pool(name="pso", bufs=2, space="PSUM") as pso,
            ):
                pts = {0: [None] * NSQ, 1: [None] * NSQ}

                # ---- up projections; most groups run as PE filler inside
                #      chunk 0's attention loop (o-proj's PSUM banks are idle
                #      there, so their pool is borrowed for the accumulators)
                def k_up(h, pl=None):
                    base = (h % 4) * 32
                    pl = pl or pso
                    ps_up = pl.tile(
                        [128, CHUNK], f32,
                        tag="ps_s" if pl is pss else "ps_o",
                        name="ps_up",
                    )
                    for blk in range(8):
                        r, half = blk // 2, blk % 2
                        src_ = kc_r if half == 0 else vc_r
                        nc.tensor.matmul(
                            ps_up[:, 0:256],
                            lhsT=wkup_sb[base : base + 32, blk * 128 : (blk + 1) * 128],
                            rhs=src_[base : base + 32, h // 4, r, :],
                            start=(blk == 0),
                            stop=(blk == 7),
                            tile_position=(base, 0),
                        )
                    nc.vector.tensor_copy(
                        kT[:, h * 256 : (h + 1) * 256], ps_up[:, 0:256]
                    )

                def v_up(tt, pl=None):
                    h, ub = tt // 2, tt % 2
                    base = (h % 4) * 32
                    pl = pl or pso
                    ps_vt = pl.tile(
                        [128, CHUNK], f32,
                        tag="ps_s" if pl is pss else "ps_o",
                        name="ps_vt",
                    )
                    for blk in range(8):
                        r, half = blk // 2, blk % 2
                        src_ = kc_r if half == 0 else vc_r
                        nc.tensor.matmul(
                            ps_vt[:, 0:128],
                            lhsT=src_[
                                base : base + 32, h // 4, 4 + r,
                                ub * 128 : (ub + 1) * 128,
                            ],
                            rhs=wvup_sb[base : base + 32, blk * 128 : (blk + 1) * 128],
                            start=(blk == 0),
                            stop=(blk == 7),
                            tile_position=(base, 0),
                        )
                    nc.vector.tensor_copy(v_sb[:, tt, 0:128], ps_vt[:, 0:128])

                # prefix: exactly what chunk 0's attention + the interleaved
                # chunk-1 h0 scores/AV need up front
                for h in range(2):
                    k_up(h, pss)
                for tt in range(4):
                    v_up(tt, pss)
                filler = (
                    [lambda h=h: k_up(h) for h in range(2, 4)]
                    + [lambda tt=tt: v_up(tt) for tt in range(4, 8)]
                    + [lambda h=h: k_up(h) for h in range(4, 8)]
                    + [lambda tt=tt: v_up(tt) for tt in range(8, NSQ)]
                )
                filler.reverse()  # consumed via .pop()

                def emit_score(c, h, j):
                    s = h % 2
                    q0 = max(0, 128 * (j - 4 * c))
                    ps_s = pss.tile([128, CHUNK], f32, tag="ps_s", name="ps_s")
                    nc.tensor.matmul(
                        ps_s[:, q0:CHUNK],
                        lhsT=kT[:, j * 128 : (j + 1) * 128],
                        rhs=qT[:, h, c * CHUNK + q0 : (c + 1) * CHUNK],
                        start=True,
                        stop=True,
                    )
                    if j >= 4 * c:
                        # causal mask on the diagonal 128-block
                        nc.vector.tensor_add(
                            ps_s[:, q0 : q0 + 128], ps_s[:, q0 : q0 + 128], mask_sb[:]
                        )
                    pt = ptpool.tile(
                        [128, CHUNK], bf16, tag=f"pt{s}_{j}", name=f"pt{s}_{j}"
                    )
                    nc.scalar.activation(pt[:, q0:CHUNK], ps_s[:, q0:CHUNK], EXP)
                    pts[s][j] = pt

                def attn_qt(c, h, qt):
                    s = h % 2
                    i = 4 * c + qt
                    qs = slice(qt * 128, (qt + 1) * 128)
                    # AV in [q, d|sum]: col 128 accumulates the softmax
                    # denominator via v_sb's ones column; normalization is a
                    # per-partition scale on the PSUM->SBUF copy
                    ps_av = psav.tile([128, 129], f32, tag="ps_av", name="ps_av")
                    for j in range(i + 1):
                        nc.tensor.matmul(
                            ps_av[:],
                            lhsT=pts[s][j][:, qs],
                            rhs=v_sb[:, j, 0:129],
                            start=(j == 0),
                            stop=(j == i),
                        )
                    rec = apool.tile([128, 1], f32, tag="rec", name="rec")
                    nc.vector.reciprocal(rec[:], ps_av[:, 128:129])
                    at_qd = apool.tile(
                        [128, 128], bf16, tag="at_qd", name="at_qd", bufs=3
                    )
                    nc.vector.tensor_scalar_mul(at_qd[:], ps_av[:, 0:128], rec[:])
                    return at_qd

                def emit_transpose(at_sb_t, h, qt, at_qd):
                    qs = slice(qt * 128, (qt + 1) * 128)
                    ps_tr = psav.tile(
                        [128, 128], bf16, tag="ps_tr", name="ps_tr", bufs=1
                    )
                    nc.tensor.transpose(ps_tr[:], at_qd[:], id_sb[:])
                    nc.vector.tensor_copy(at_sb_t[:, h, qs], ps_tr[:])

                def o_pair(at_sb_t, i, pn, split=False):
                    out_sb = opool.tile(
                        [128, 2 * CHUNK], bf16, tag="out_sb", name="out_sb"
                    )
                    for half in range(2):
                        n = 2 * pn + half
                        ps_o = pso.tile([128, CHUNK], f32, tag="ps_o", name="ps_o")
                        for hh in range(HPC):
                            nc.tensor.matmul(
                                ps_o[:],
                                lhsT=at_sb_t[:, hh, (i % 4) * 128 : (i % 4 + 1) * 128],
                                rhs=wo_sb[:, hh, n * CHUNK : (n + 1) * CHUNK],
                                start=(hh == 0),
                                stop=(hh == HPC - 1),
                            )
                        if split and half == 1:
                            nc.vector.tensor_copy(
                                out_sb[:, CHUNK : CHUNK + 256], ps_o[:, 0:256]
                            )
                            nc.scalar.copy(
                                out_sb[:, CHUNK + 256 : 2 * CHUNK], ps_o[:, 256:512]
                            )
                        else:
                            nc.vector.tensor_copy(
                                out_sb[:, half * CHUNK : (half + 1) * CHUNK], ps_o[:]
                            )
                        if split and half == 0:
                            nc.sync.dma_start(
                                out=outp[
                                    i * 128 : (i + 1) * 128,
                                    n * CHUNK : (n + 1) * CHUNK,
                                ],
                                in_=out_sb[:, 0:CHUNK],
                            )
                    if not split:
                        (nc.sync if pn % 2 == 0 else nc.gpsimd).dma_start(
                            out=outp[
                                i * 128 : (i + 1) * 128,
                                2 * pn * CHUNK : 2 * (pn + 1) * CHUNK,
                            ],
                            in_=out_sb[:],
                        )
                    else:
                        nc.scalar.dma_start(
                            out=outp[
                                i * 128 : (i + 1) * 128,
                                (2 * pn + 1) * CHUNK : (2 * pn + 1) * CHUNK + 256,
                            ],
                            in_=out_sb[:, CHUNK : CHUNK + 256],
                        )
                        nc.sync.dma_start(
                            out=outp[
                                i * 128 : (i + 1) * 128,
                                (2 * pn + 1) * CHUNK + 256 : 2 * (pn + 1) * CHUNK,
                            ],
                            in_=out_sb[:, CHUNK + 256 : 2 * CHUNK],
                        )

                prev_at = None  # at_sb tile of the previous chunk
                for c in range(NCHUNK):
                    nj = 4 * c + 4
                    at_sb = apool.tile(
                        [128, HPC, CHUNK], bf16, tag="at_sb", name="at_sb"
                    )
                    if c == 0:
                        for j in range(nj):
                            emit_score(0, 0, j)
                    slot = 0
                    for h in range(HPC):
                        if h + 1 < HPC:
                            njs = list(range(nj))
                            nxt = (c, h + 1)
                        elif c + 1 < NCHUNK:
                            njs = list(range(4 * (c + 1) + 4))
                            nxt = (c + 1, 0)
                        else:
                            njs, nxt = [], None
                        pending = None
                        for qt in range(4):
                            at_qd = attn_qt(c, h, qt)
                            lo = (qt * len(njs)) // 4
                            hi = ((qt + 1) * len(njs)) // 4
                            for j in njs[lo:hi]:
                                emit_score(nxt[0], nxt[1], j)
                            if prev_at is not None:
                                o_pair(prev_at, 4 * (c - 1) + slot // 4, slot % 4)
                            elif filler:
                                filler.pop()()
                                if len(filler) > 15 - slot:
                                    filler.pop()()
                            slot += 1
                            if pending is not None:
                                emit_transpose(at_sb, h, pending[0], pending[1])
                            pending = (qt, at_qd)
                        emit_transpose(at_sb, h, pending[0], pending[1])
                    prev_at = at_sb
                # last chunk's o-projection runs solo
                for qo in range(4):
                    for pn in range(4):
                        o_pair(
                            prev_at,
                            4 * (NCHUNK - 1) + qo,
                            pn,
                            split=(qo == 3 and pn == 3),
                        )
    nc.compile()
    return nc


def _prep_inputs(hidden_states, Wq, Wk_down, Wv_down, Wk_up, Wv_up, Wo):
    bf = ml_dtypes.bfloat16
    hs = np.asarray(hidden_states, dtype=np.float32).reshape(S, HID)
    ht = np.ascontiguousarray(hs.T).astype(bf)
    scale = np.float32(1.0) / np.sqrt(np.float32(D))
    Wq = np.asarray(Wq, dtype=np.float32)
    Wo = np.asarray(Wo, dtype=np.float32)
    wkdt = np.ascontiguousarray(np.asarray(Wk_down, np.float32).T).astype(bf)
    wvdt = np.ascontiguousarray(np.asarray(Wv_down, np.float32).T).astype(bf)
    # transposed causal mask: rows t, cols q; allowed where q >= t
    mask = np.where(
        np.arange(128)[None, :] >= np.arange(128)[:, None], 0.0, NEG
    ).astype(np.float32)
    identity = np.eye(128, dtype=bf)

    def up_blocks(w):  # w: (128, 256) rows of Wk_up/Wv_up for this core
        arr = np.zeros((128, 8 * 128), np.float32)
        for r in range(4):
            for half in range(2):
                blk = r * 2 + half
                bT = w[:, r * 64 + half * 32 : r * 64 + half * 32 + 32].T
                for b in range(4):
                    arr[b * 32 : (b + 1) * 32, blk * 128 : (blk + 1) * 128] = bT
        return arr.astype(bf)

    in_maps = []
    for c in range(NCORE):
        htm = np.ascontiguousarray(ht[:, c * SSH : (c + 1) * SSH])
        wqt = np.ascontiguousarray((Wq[c * 512 : (c + 1) * 512, :] * scale).T).astype(
            bf
        )
        wkup = up_blocks(np.asarray(Wk_up[c * 128 : (c + 1) * 128, :], np.float32))
        wvup = up_blocks(np.asarray(Wv_up[c * 128 : (c + 1) * 128, :], np.float32))
        wot = np.ascontiguousarray(Wo[:, c * 512 : (c + 1) * 512].T).astype(bf)
        in_maps.append(
            dict(
                ht=ht,
                htm=htm,
                wqt=wqt,
                wkdt=wkdt,
                wvdt=wvdt,
                wkup=wkup,
                wvup=wvup,
                wot=wot,
                maskt=mask,
                ident=identity,
            )
        )
    return in_maps


def run(trace=False, **inputs):
    from concourse.bass_utils import run_bass_kernel_spmd

    global _BUILT
    if _BUILT is None:
        _BUILT = _build()
    in_maps = _prep_inputs(**inputs)
    res = run_bass_kernel_spmd(
        _BUILT, in_maps, core_ids=list(range(NCORE)), trace=trace
    )
    acc = np.array(res.results[0]["out"], dtype=np.float32, copy=True)
    for r in res.results[1:]:
        acc += np.asarray(r["out"], dtype=np.float32)
    return acc.reshape(B, S, HID), res


def kernel(**inputs):
    out, _ = run(trace=False, **inputs)
    return out
